# revision 1
# baseline (speedup 1.0000x reference)
"""Block-diagonal linear (BlockLinear) Trainium2 Bass kernel.

Problem: out[b, n, o] = sum_i x[b, n, i] * W[n, o, i] + bias[n, o]
  x: [1024, 1024, 64] f32, W: [1024, 64, 64] f32, bias: [1024, 64] f32

Sharding: block-parallel over n (num_blocks) across 8 NeuronCores;
each core owns 128 blocks. No inter-core communication.

Per-core algorithm (all fp32):
  - The contraction dim i is innermost in DRAM, so x tiles arrive in
    SBUF as [b=128 partitions, i free]. The tensor engine contracts over
    the partition dim, so x is transposed on chip: a PE transpose
    (x_tile.T @ I) over a [128b, 128] tile covering TWO blocks
    (2 x 64 = 128) yields xT [i2=128, b=128] in PSUM at full array width.
  - Weights are expanded on chip into block-pair block-diagonal tiles
    W2[pair] = [[W[2p].T, 0], [0, W[2p+1].T]]  (shape [128, 128]),
    so a single fp32 matmul  xT.T @ W2  = [b=128, o2=128] computes two
    blocks at once with K=128 (full partition utilization). Only the
    compact 2MB W.T is DMA'd; zeros + layout are built by DVE.
  - Bias is DMA'd compact (32KB), broadcast across partitions on chip by
    a PE ones-outer-product, and added by the DVE during the PSUM->SBUF
    copy of the output.
  - All DRAM<->SBUF DMAs move >=2KB contiguous per partition (line rate).
  - x reads ride the sync HWDGE ring; out writes + constants ride the
    scalar HWDGE ring so neither stream queues behind the other.

The kernel is memory-bound: per core it streams 32MB of x in and 32MB of
out at the measured ~300GB/s/core mixed R/W rate (~220us floor measured
for a pure-DMA loop with this access pattern); PE transposes/matmuls,
ACT copies, and DVE adds hide underneath (~231us measured end to end).
"""

import contextlib

import numpy as np

import concourse.bass as bass
import concourse.bacc as bacc
import concourse.tile as tile
from concourse import mybir
from concourse.bass_utils import run_bass_kernel_spmd

F32 = mybir.dt.float32

B = 1024          # batch
NB = 1024         # num_blocks (total)
DIN = 64
DOUT = 64
NCORES = 8
NB_C = NB // NCORES          # 128 blocks per core
CHUNK = 128                  # batch rows per tile (SBUF partitions)
NCHUNK = B // CHUNK          # 8
XH = 64                      # blocks per x DMA (16KB/partition)
OB = 32                      # blocks per out DMA (8KB/partition)
GRP = 8                      # blocks per PSUM bank group


def build_program(n_reps=1, xh=XH, ob=OB, pt_bufs=4, po_bufs=2,
                  xt_bufs=8, x_bufs=3, o_bufs=3, plain_mm_transpose=False,
                  out_engine="scalar", split_first=8):
    """n_reps>1 wraps the main loop in a HW loop repeating the whole
    computation — used only for timing (amortizes dispatch overhead)."""
    nc = bacc.Bacc(
        "TRN2", target_bir_lowering=False, debug=False, num_devices=NCORES
    )
    x_d = nc.dram_tensor("x", [B, NB_C, DIN], F32, kind="ExternalInput")
    # compact stacked W.T: rows 0:64 = W[2p].T, rows 64:128 = W[2p+1].T
    w2c_d = nc.dram_tensor("w2c", [128, NB_C // 2, DOUT], F32,
                           kind="ExternalInput")
    bc_d = nc.dram_tensor("bc", [1, NB_C * DOUT], F32, kind="ExternalInput")
    id_d = nc.dram_tensor("ident", [128, 128], F32, kind="ExternalInput")
    o_d = nc.dram_tensor("out", [B, NB_C, DOUT], F32, kind="ExternalOutput")

    xa, w2ca, bca, ida, oa = (t.ap() for t in (x_d, w2c_d, bc_d, id_d, o_d))

    with tile.TileContext(nc) as tc:
        with (
            tc.tile_pool(name="const", bufs=1) as cpool,
            tc.tile_pool(name="xin", bufs=x_bufs) as xpool,
            tc.tile_pool(name="xs", bufs=1) as xspool,
            tc.tile_pool(name="xt", bufs=xt_bufs) as xtpool,
            tc.tile_pool(name="pt", bufs=pt_bufs, space="PSUM") as ptpool,
            tc.tile_pool(name="po", bufs=po_bufs, space="PSUM") as popool,
            tc.tile_pool(name="oo", bufs=o_bufs) as opool,
        ):
            ident = cpool.tile([128, 128], F32)
            nc.sync.dma_start(ident[:], ida[:])

            # Constants ride the scalar HWDGE ring so the sync ring's FIFO
            # leads with the first x tiles (compute starts sooner).
            # --- on-chip W2 block-diagonal expansion (saves 2MB DMA) ---
            w2 = cpool.tile([128, NB_C // 2, 128], F32)
            w2c = xpool.tile([128, NB_C // 2, DOUT], F32, tag="x_t")  # borrow slot
            nc.scalar.dma_start(w2c[:], w2ca[:])
            nc.gpsimd.memset(w2[:], 0.0)
            nc.vector.tensor_copy(w2[0:64, :, 0:64], w2c[0:64, :, :])
            nc.vector.tensor_copy(w2[64:128, :, 64:128], w2c[64:128, :, :])

            # --- on-chip bias broadcast (saves 4MB DMA) ---
            # ones[1,128].T @ bias[1,512] on the (idle-at-startup) PE
            # replicates bias across partitions without touching the SDMA
            # engines the x-read fill is using.
            bias_c = cpool.tile([1, NB_C * DOUT], F32)
            nc.scalar.dma_start(bias_c[:], bca[:])
            ones = cpool.tile([1, 128], F32)
            nc.gpsimd.memset(ones[:], 1.0)
            bb = cpool.tile([128, NB_C // GRP, GRP, DOUT], F32)
            for g in range(NB_C // GRP):
                pb = popool.tile([CHUNK, GRP, DOUT], F32, tag="po")
                nc.tensor.matmul(
                    pb[:], ones[:], bias_c[:, g * GRP * DOUT:(g + 1) * GRP * DOUT],
                    start=True, stop=True,
                )
                nc.vector.tensor_copy(bb[:, g, :, :], pb[:])

            rep_cm = (
                tc.For_i(0, n_reps, 1) if n_reps > 1 else contextlib.nullcontext()
            )
            with rep_cm:
                main_body(nc, tc, xa, oa, w2, bb, ident,
                          xpool, xspool, xtpool, ptpool, popool, opool,
                          xh=xh, ob_sz=ob, plain_mm_transpose=plain_mm_transpose,
                          out_engine=out_engine, split_first=split_first)

    nc.compile()
    return nc


def main_body(nc, tc, xa, oa, w2, bb, ident,
              xpool, xspool, xtpool, ptpool, popool, opool,
              xh=XH, ob_sz=OB, plain_mm_transpose=False, out_engine="sync",
              split_first=8):
    wr = getattr(nc, out_engine)
    for c in range(NCHUNK):
        for h in range(NB_C // xh):
            ramp = c == 0 and h == 0 and split_first > 0
            x_t = xpool.tile([CHUNK, xh, DIN], F32, tag="x_t")
            if ramp:
                # Ramp-up: the first blocks land as their own small tile so
                # the first transposes wait on a 256KB DMA, not a 2MB one.
                x_small = xspool.tile([CHUNK, split_first, DIN], F32)
                nc.sync.dma_start(x_small[:], xa[0:CHUNK, 0:split_first, :])
                nc.sync.dma_start(
                    x_t[:, split_first:, :],
                    xa[0:CHUNK, split_first:xh, :],
                )
            else:
                nc.sync.dma_start(
                    x_t[:],
                    xa[c * CHUNK:(c + 1) * CHUNK, h * xh:(h + 1) * xh, :],
                )
            last_tile = c == NCHUNK - 1 and h == NB_C // xh - 1
            for ob in range(xh // ob_sz):
                # Drain: the final out tile is written per 8-block group so
                # the kernel tail is a 256KB DMA, not a 1MB one.
                fine = last_tile and ob == xh // ob_sz - 1
                o_t = None if fine else opool.tile([CHUNK, ob_sz, DOUT], F32)
                for gi in range(ob_sz // GRP):
                    blk0 = h * xh + ob * ob_sz + gi * GRP
                    g = blk0 // GRP
                    po = popool.tile([CHUNK, GRP, DOUT], F32)
                    for q in range(GRP // 2):
                        pair = blk0 // 2 + q
                        xoff = ob * ob_sz + gi * GRP + 2 * q
                        if ramp and xoff < split_first:
                            src = x_small[:, xoff:xoff + 2, :]
                        else:
                            src = x_t[:, xoff:xoff + 2, :]
                        pt = ptpool.tile([128, CHUNK], F32)
                        if plain_mm_transpose:
                            nc.tensor.matmul(
                                pt[:], src, ident[:],
                                start=True, stop=True,
                            )
                        else:
                            nc.tensor.transpose(pt[:], src, ident[:])
                        xts = xtpool.tile([128, CHUNK], F32)
                        nc.scalar.mul(xts[:], pt[:], 1.0)
                        nc.tensor.matmul(
                            po[:, 2 * q:2 * q + 2, :],
                            xts[:],
                            w2[:, pair, :],
                            start=True,
                            stop=True,
                        )
                    if fine:
                        o_small = opool.tile([CHUNK, GRP, DOUT], F32,
                                             tag="o_fine")
                        nc.vector.tensor_add(
                            o_small[:], po[:], bb[:, g, :, :],
                        )
                        nb0 = h * xh + ob * ob_sz + gi * GRP
                        wr.dma_start(
                            oa[c * CHUNK:(c + 1) * CHUNK, nb0:nb0 + GRP, :],
                            o_small[:],
                        )
                    else:
                        nc.vector.tensor_add(
                            o_t[:, gi * GRP:(gi + 1) * GRP, :],
                            po[:],
                            bb[:, g, :, :],
                        )
                if not fine:
                    nb0 = h * xh + ob * ob_sz
                    wr.dma_start(
                        oa[c * CHUNK:(c + 1) * CHUNK, nb0:nb0 + ob_sz, :],
                        o_t[:],
                    )


_PROGRAMS = {}


def get_program(n_reps=1):
    if n_reps not in _PROGRAMS:
        _PROGRAMS[n_reps] = build_program(n_reps)
    return _PROGRAMS[n_reps]


def prep_core_inputs(x, W, b, core):
    """Host-side shard + layout prep for one core."""
    n0, n1 = core * NB_C, (core + 1) * NB_C
    xs = np.ascontiguousarray(x[:, n0:n1, :], dtype=np.float32)
    Wk = W[n0:n1]                                  # [128, 64, 64] (n, o, i)
    WT = Wk.transpose(0, 2, 1)                     # [128, 64, 64] (n, i, o)
    # compact stacked layout [i2=128, pair, o]: rows 0:64 even blocks,
    # rows 64:128 odd blocks
    w2c = np.empty((128, NB_C // 2, DOUT), dtype=np.float32)
    w2c[:64] = WT[0::2].transpose(1, 0, 2)
    w2c[64:] = WT[1::2].transpose(1, 0, 2)
    bc = np.ascontiguousarray(b[n0:n1].reshape(1, NB_C * DOUT),
                              dtype=np.float32)
    ident = np.eye(128, dtype=np.float32)
    return {"x": xs, "w2c": w2c, "bc": bc, "ident": ident}


def make_in_maps(x, W, b):
    return [prep_core_inputs(x, W, b, k) for k in range(NCORES)]


def kernel(x, W, b):
    nc = get_program()
    in_maps = make_in_maps(x, W, b)
    res = run_bass_kernel_spmd(nc, in_maps, list(range(NCORES)))
    out = np.concatenate([res.results[k]["out"] for k in range(NCORES)], axis=1)
    return out



# revision 4
# speedup vs baseline: 2.1342x; 2.1342x over previous
"""Block-diagonal linear (BlockLinear) Trainium2 Bass kernel, v2.

Problem: out[b, n, o] = sum_i x[b, n, i] * W[n, o, i] + bias[n, o]
  x: [1024, 1024, 64] f32, W: [1024, 64, 64] f32, bias: [1024, 64] f32

Sharding: block-parallel over n (num_blocks) across 8 NeuronCores;
each core owns 128 blocks. No inter-core communication.

The kernel is HBM-bound (per-NC HBM limit ~358 GB/s), so v2 cuts the
wire format to fp16 (rel err ~5e-4, gate is 2e-2): x is cast host-side
to fp16 and uploaded in its NATURAL [b, n, i] layout; the output comes
back fp16 [b, n, o] and is cast to f32 host-side. 33MB/core on the
wire vs 66MB for the f32 baseline.

Per-core pipeline (128 blocks = 64 block-pairs):
  - x transposition (contraction dim i must sit on SBUF partitions) is
    done by the DMA XBAR: one dma_start(transpose=True) per block-pair
    reads x[:, 2p:2p+2, :] as [1024 b, 128 (n,i)] and lands
    xT [i2=128, b=1024] fp16 in SBUF. No PE transposes (the f32
    baseline burned ~140us of PE there), no host transpose (1 CPU).
  - Weights are expanded on chip into block-pair block-diagonal tiles
    W2[pair] = [[W[2p].T, 0], [0, W[2p+1].T]] (fp16 [128, 128]), so
    matmul(po, lhsT=xT[:, chunk], rhs=W2[pair]) = [b=128, o2=128]
    computes two blocks at K=128 full array width.
  - Bias is broadcast across partitions once via a K=1 ones matmul
    into bb [128, pair, o2] f32; DVE/gpsimd tensor_add fuse the
    PSUM->SBUF drain, the bias add, and the f32->fp16 cast.
  - Reads (XBAR transposes) ride the sync HWDGE ring; writes + consts
    ride the scalar ring, so the two streams don't queue behind each
    other and overlap under the shared HBM cap.
"""

import contextlib

import numpy as np

import concourse.bass as bass
import concourse.bacc as bacc
import concourse.tile as tile
from concourse import mybir
from concourse.bass_utils import run_bass_kernel_spmd

F32 = mybir.dt.float32
F16 = mybir.dt.float16

B = 1024          # batch
NB = 1024         # num_blocks (total)
DIN = 64
DOUT = 64
NCORES = 8
NB_C = NB // NCORES          # 128 blocks per core
NPAIR = NB_C // 2            # 64 block-pairs per core
CHUNK = 128                  # batch rows per matmul output tile
NCHUNK = B // CHUNK          # 8
SLAB = 16                    # block-pairs per x-transpose slab
GRP = 4                      # pairs per PSUM bank ([128, 4*128] f32)


def build_program(n_reps=1, slab=SLAB, grp=GRP, x_bufs=2, o_bufs=10,
                  po_bufs=6, gpsimd_grp=4):
    """n_reps>1 wraps the main loop in a HW loop repeating the whole
    computation - used only for timing (amortizes dispatch overhead)."""
    nc = bacc.Bacc(
        "TRN2", target_bir_lowering=False, debug=False, num_devices=NCORES
    )
    x_d = nc.dram_tensor("x", [B, NB_C, DIN], F16, kind="ExternalInput")
    # compact stacked W.T: rows 0:64 = W[2p].T, rows 64:128 = W[2p+1].T
    w2c_d = nc.dram_tensor("w2c", [128, NPAIR, DOUT], F16,
                           kind="ExternalInput")
    bc_d = nc.dram_tensor("bc", [1, NB_C * DOUT], F32, kind="ExternalInput")
    o_d = nc.dram_tensor("out", [B, NB_C, DOUT], F16, kind="ExternalOutput")

    xa, w2ca, bca, oa = (t.ap() for t in (x_d, w2c_d, bc_d, o_d))

    with tile.TileContext(nc) as tc:
        with (
            tc.tile_pool(name="const", bufs=1) as cpool,
            tc.tile_pool(name="xt", bufs=x_bufs) as xpool,
            tc.tile_pool(name="oo", bufs=o_bufs) as opool,
            tc.tile_pool(name="po", bufs=po_bufs, space="PSUM") as popool,
            tc.tile_pool(name="pb", bufs=2, space="PSUM") as pbpool,
        ):
            # --- on-chip W2 block-diagonal expansion (halves W DMA) ---
            w2 = cpool.tile([128, NPAIR, 128], F16)
            w2c = cpool.tile([128, NPAIR, DOUT], F16)
            nc.scalar.dma_start(w2c[:], w2ca[:])
            nc.gpsimd.memset(w2[:], 0.0)
            nc.vector.tensor_copy(w2[0:64, :, 0:64], w2c[0:64, :, :])
            nc.vector.tensor_copy(w2[64:128, :, 64:128], w2c[64:128, :, :])

            # --- bias broadcast across partitions: ones[1,128].T @ bc ---
            bias_c = cpool.tile([1, NB_C * DOUT], F32)
            nc.scalar.dma_start(bias_c[:], bca[:])
            ones = cpool.tile([1, 128], F32)
            nc.gpsimd.memset(ones[:], 1.0)
            bb = cpool.tile([128, NPAIR, 128], F32)
            for t in range(NPAIR // 4):
                pb = pbpool.tile([128, 4, 128], F32)
                nc.tensor.matmul(
                    pb[:], ones[:], bias_c[:, t * 512:(t + 1) * 512],
                    start=True, stop=True,
                )
                nc.vector.tensor_copy(bb[:, 4 * t:4 * t + 4, :], pb[:])

            rep_cm = (
                tc.For_i(0, n_reps, 1) if n_reps > 1 else contextlib.nullcontext()
            )
            with rep_cm:
                main_body(nc, tc, xa, oa, w2, bb, xpool, opool, popool,
                          slab=slab, grp=grp, gpsimd_grp=gpsimd_grp)

    nc.compile()
    return nc


def main_body(nc, tc, xa, oa, w2, bb, xpool, opool, popool,
              slab=SLAB, grp=GRP, gpsimd_grp=3):
    for s in range(NPAIR // slab):
        xt = xpool.tile([128, slab, B], F16)
        for p in range(slab):
            n0 = (s * slab + p) * 2
            nc.sync.dma_start(xt[:, p, :], xa[:, n0:n0 + 2, :],
                              transpose=True)
        for c in range(NCHUNK):
            ot = opool.tile([CHUNK, slab, 128], F16)
            for g in range(slab // grp):
                po = popool.tile([CHUNK, grp, 128], F32)
                for q in range(grp):
                    p = g * grp + q
                    pair = s * slab + p
                    nc.tensor.matmul(
                        po[:, q, :],
                        xt[:, p, c * CHUNK:(c + 1) * CHUNK],
                        w2[:, pair, :],
                        start=True, stop=True,
                    )
                eng = nc.gpsimd if g >= gpsimd_grp else nc.vector
                eng.tensor_add(
                    ot[:, g * grp:(g + 1) * grp, :],
                    po[:],
                    bb[:, s * slab + g * grp:s * slab + (g + 1) * grp, :],
                )
            nc.scalar.dma_start(
                oa[c * CHUNK:(c + 1) * CHUNK,
                   s * slab * 2:(s + 1) * slab * 2, :],
                ot[:],
            )


_PROGRAMS = {}


def get_program(n_reps=1):
    if n_reps not in _PROGRAMS:
        _PROGRAMS[n_reps] = build_program(n_reps)
    return _PROGRAMS[n_reps]


def prep_core_inputs(xh, W, b, core):
    """Host-side shard + layout prep for one core (no transposes of x -
    the DMA XBAR transposes on chip; host only casts and slices)."""
    n0, n1 = core * NB_C, (core + 1) * NB_C
    xs = np.ascontiguousarray(xh[:, n0:n1, :])
    Wk = W[n0:n1]                                  # [128, 64, 64] (n, o, i)
    WT = Wk.transpose(0, 2, 1).astype(np.float16)  # [128, 64, 64] (n, i, o)
    # compact stacked layout [i2=128, pair, o]: rows 0:64 even blocks,
    # rows 64:128 odd blocks
    w2c = np.empty((128, NPAIR, DOUT), dtype=np.float16)
    w2c[:64] = WT[0::2].transpose(1, 0, 2)
    w2c[64:] = WT[1::2].transpose(1, 0, 2)
    # bias, pair-interleaved: row p = [b[2p], b[2p+1]]
    bc = np.ascontiguousarray(
        b[n0:n1].reshape(1, NB_C * DOUT), dtype=np.float32)
    return {"x": xs, "w2c": w2c, "bc": bc}


def make_in_maps(x, W, b):
    xh = np.asarray(x, dtype=np.float16)
    return [prep_core_inputs(xh, W, b, k) for k in range(NCORES)]


def kernel(x, W, b):
    nc = get_program()
    in_maps = make_in_maps(x, W, b)
    res = run_bass_kernel_spmd(nc, in_maps, list(range(NCORES)))
    out = np.concatenate(
        [res.results[k]["out"].astype(np.float32) for k in range(NCORES)],
        axis=1,
    )
    return out
